# revision 23
# baseline (speedup 1.0000x reference)
"""Causal self-attention (B=4, S=2048, E=2048, H=16, D=128) on 8 TRN2 cores.

Sharding: batch (4-way) x head-halves (2-way) -> 8 cores.
Core c handles batch b = c//2, heads p*8..p*8+8 where p = c%2.

Per-core kernel (single NEFF, SPMD):
  Phase 1: QKV projection.  x^T resident in SBUF [E,S]; per head computes
           q^T, k^T in [D,S] layout (matmul lhsT = W columns, rhs = x^T) and
           v in [S,D] layout (lhsT = x^T tile, rhs = W_v columns).
           q^T -> DRAM scratch, k^T -> output, v -> output.
  Phase 2: attention + proj in one pass; Wproj resident, k^T / v / q^T
           streamed per (q-tile, head).
           Scores computed transposed: ST[k,q] = (kT chunk) lhsT vs qT rhs.
           exp on scalar engine (no max subtraction needed: scores ~ N(0,1)),
           row sums accumulated on the vector engine + one ones-vector
           matmul, y^T via lhsT=v rhs=P^T, normalization via PE
           outer-product broadcast of 1/sums, out = y @ Wproj accumulated
           over all 8 heads in PSUM.  Normalization work is software-
           pipelined two heads deep to keep it off PE's critical path.
Host: shards inputs, sums the 2 partial outs per batch, fixes up k/v biases.

All DRAM tensors are laid out so DMA transfers are large contiguous blocks
(weights pre-packed on host into [head, chunk, 128, width] form).

Matmul inputs use float32r (single-pass PE, mantissa rounded to 12 bits)
unless CK_MM_DT=f32 (2-pass full fp32, ~4x slower).
"""

import math
import os
import sys

for _p in ("/opt/trn_rl_repo",):
    if os.path.isdir(_p) and _p not in sys.path:
        sys.path.append(_p)

import numpy as np

import concourse.bacc as bacc
import concourse.mybir as mybir
import concourse.tile as tile
from concourse.bass_utils import run_bass_kernel_spmd

B, S, E, H = 4, 2048, 2048, 16
D = E // H            # 128
P = 128               # partitions
HPC = H // 2          # heads per core = 8
NE = E // P           # 16 e-chunks
NS = S // P           # 16 s-chunks
STQ = 512             # phase-2 q tile width
NR = S // STQ         # 4 q tiles
NN = E // 512         # 4 output col chunks
NORM = 1.0 / math.sqrt(D)
NEG = -1.0e30

F32 = mybir.dt.float32
F32R = mybir.dt.float32r

MMDT = {"f32": F32, "f32r": F32R}[os.environ.get("CK_MM_DT", "f32r")]


def _build_program():
    nc = bacc.Bacc("TRN2", target_bir_lowering=False, debug=False)
    Exp = mybir.ActivationFunctionType.Exp
    Ident = mybir.ActivationFunctionType.Identity
    mm = nc.tensor.matmul

    with tile.TileContext(nc) as tc, \
         tc.tile_pool(name="dram", bufs=1, space="DRAM") as dram, \
         tc.tile_pool(name="const", bufs=1) as cp:

        def din(name, shape, dt=MMDT):
            return dram.tile(shape, dt, kind="ExternalInput", name=name,
                             uniquify=False)

        def dout(name, shape, dt=F32):
            return dram.tile(shape, dt, kind="ExternalOutput", name=name,
                             uniquify=False)

        xT_d = din("xT", [E, S])
        wq_d = din("wq", [HPC, P, NE, D])
        wk_d = din("wk", [HPC, P, NE, D])
        wv_d = din("wv", [2, NE, P, 512])
        wp_d = din("wp", [NN, HPC, P, 512])
        bq_d = din("bq", [P, HPC], F32)
        kT_d = dout("kT_out", [HPC, NR, P, 512], MMDT)
        v_d = dout("v_out", [HPC, P, NS, D], MMDT)
        o_d = dout("out_p", [S, E])
        qT_d = dram.tile([HPC, NR, P, 512], MMDT, kind="Internal",
                         name="qT_s", uniquify=False)

        # ---------------- constants ----------------
        masks = []
        for m in range(4):
            mk = cp.tile([P, STQ], F32, name=f"mask{m}", tag=f"mask{m}")
            nc.gpsimd.memset(mk, 0.0)
            # ST layout [k, q]: keep where q - k - 128*m >= 0
            nc.gpsimd.affine_select(
                out=mk, in_=mk, compare_op=mybir.AluOpType.is_ge,
                fill=NEG, base=-P * m, channel_multiplier=-1,
                pattern=[[1, STQ]])
            masks.append(mk)
        ones_col_f = cp.tile([P, 1], F32, name="ones_col_f", tag="oncf")
        nc.gpsimd.memset(ones_col_f, 1.0)
        ones_col = cp.tile([P, 1], MMDT, name="ones_col", tag="onc")
        nc.scalar.copy(ones_col, ones_col_f)
        ones_row_f = cp.tile([1, P], F32, name="ones_row_f", tag="onrf")
        nc.gpsimd.memset(ones_row_f, 1.0)
        ones_row = cp.tile([1, P], MMDT, name="ones_row", tag="onr")
        nc.scalar.copy(ones_row, ones_row_f)
        bq_sb = cp.tile([P, HPC], F32, name="bq_sb", tag="bq")
        nc.sync.dma_start(bq_sb, bq_d)

        # ---------------- phase 1: QKV ----------------
        with tc.tile_pool(name="p1", bufs=1) as p1, \
             tc.tile_pool(name="psP", bufs=8, space="PSUM") as psP:
            xt = p1.tile([P, NE, S], MMDT, name="xt", tag="xt")
            for c in range(NE):
                nc.sync.dma_start(xt[:, c, :], xT_d[c * P:(c + 1) * P, :])

            # q^T and k^T per head, interleaved per e-chunk
            with tc.tile_pool(name="ws", bufs=3) as ws, \
                 tc.tile_pool(name="evq", bufs=3) as evp:
                for h in range(HPC):
                    wtq = ws.tile([P, NE, D], MMDT, name=f"wq{h}", tag="w")
                    nc.sync.dma_start(wtq, wq_d[h])
                    wtk = ws.tile([P, NE, D], MMDT, name=f"wk{h}", tag="w")
                    nc.sync.dma_start(wtk, wk_d[h])
                    psq = [psP.tile([P, 512], F32, name=f"psq{h}_{n}",
                                    tag="ps") for n in range(4)]
                    psk = [psP.tile([P, 512], F32, name=f"psk{h}_{n}",
                                    tag="ps") for n in range(4)]
                    for c in range(NE):
                        for n in range(4):
                            mm(psq[n], lhsT=wtq[:, c, :],
                               rhs=xt[:, c, n * 512:(n + 1) * 512],
                               start=(c == 0), stop=(c == NE - 1))
                        for n in range(4):
                            mm(psk[n], lhsT=wtk[:, c, :],
                               rhs=xt[:, c, n * 512:(n + 1) * 512],
                               start=(c == 0), stop=(c == NE - 1))
                    evq_t = evp.tile([P, NR, 512], MMDT,
                                     name=f"evq{h}", tag="ev")
                    evk_t = evp.tile([P, NR, 512], MMDT,
                                     name=f"evk{h}", tag="ev")
                    for n in range(4):
                        nc.scalar.activation(evq_t[:, n, :], psq[n], Ident,
                                             bias=bq_sb[:, h:h + 1], scale=1.0)
                        nc.scalar.copy(evk_t[:, n, :], psk[n])
                    nc.sync.dma_start(
                        qT_d[h].rearrange("n p s -> p n s"), evq_t)
                    nc.sync.dma_start(
                        kT_d[h].rearrange("n p s -> p n s"), evk_t)

            # v in [S, 4*D] groups of 4 heads (wv resident per group)
            with tc.tile_pool(name="wvp", bufs=18) as wvp, \
                 tc.tile_pool(name="evv", bufs=3) as evvp:
                for g4 in range(2):
                    wvt = []
                    for c in range(NE):
                        wvc = wvp.tile([P, 512], MMDT,
                                       name=f"wv{g4}_{c}", tag="wv")
                        nc.sync.dma_start(wvc, wv_d[g4, c])
                        wvt.append(wvc)
                    for st in range(NS):
                        pv = psP.tile([P, 512], F32,
                                      name=f"pv{g4}_{st}", tag="ps")
                        for c in range(NE):
                            mm(pv, lhsT=xt[:, c, st * P:(st + 1) * P],
                               rhs=wvt[c], start=(c == 0), stop=(c == NE - 1))
                        ev = evvp.tile([P, 512], MMDT,
                                       name=f"evv{g4}_{st}", tag="evv")
                        nc.scalar.copy(ev, pv)
                        for hh in range(4):
                            nc.sync.dma_start(
                                v_d[g4 * 4 + hh][:, st, :],
                                ev[:, hh * D:(hh + 1) * D])

        # ---------------- phase 2a: attention (head-outer) ----------------
        y_d = dram.tile([HPC, NR, P, 512], MMDT, kind="Internal",
                        name="y_s", uniquify=False)
        from contextlib import ExitStack
        with ExitStack() as es:
            pool = lambda nm, bufs, **kw: es.enter_context(
                tc.tile_pool(name=nm, bufs=bufs, **kw))
            wpp = pool("wpp", 1); ktp = pool("ktp", 3); vtp = pool("vtp", 2)
            qtp = pool("qtp", 3); ptp = pool("ptp", 4); tmpp = pool("tmpp", 1)
            accp = pool("accp", 2); bcp = pool("bcp", 1); smp = pool("smp", 2)
            ytev = pool("ytev", 3); ysp = pool("ysp", 8); obp = pool("obp", 3)
            psX = pool("psX", 3, space="PSUM")
            psY = pool("psY", 3, space="PSUM")
            psS = pool("psS", 2, space="PSUM")

            wpt = wpp.tile([P, NN, HPC, 512], MMDT, name="wpt", tag="wp")

            state = {}
            pend = []

            def finalize_a(key):
                acc_ab, yt_ps = state[key]
                sm_ps = psS.tile([1, STQ], F32, name=f"sm_{key}", tag="sm")
                mm(sm_ps, lhsT=ones_col, rhs=acc_ab[0], start=True, stop=False)
                mm(sm_ps, lhsT=ones_col, rhs=acc_ab[1], start=False, stop=True)
                sm_sb = smp.tile([1, STQ], MMDT, name=f"smb_{key}", tag="smb")
                nc.scalar.copy(sm_sb, sm_ps)
                state[key] = (yt_ps, sm_sb)

            def finalize_b(key):
                h, r = key
                yt_ps, sm_sb = state.pop(key)
                bc_ps = psX.tile([P, STQ], F32, name=f"bcp_{key}", tag="x")
                mm(bc_ps, lhsT=ones_row, rhs=sm_sb, start=True, stop=True)
                bc_sb = bcp.tile([P, STQ], F32, name=f"bc_{key}", tag="bc")
                nc.vector.reciprocal(bc_sb, bc_ps)
                ytn = ytev.tile([P, STQ], MMDT, name=f"ytn_{key}", tag="ytn")
                nc.vector.tensor_mul(ytn, yt_ps, bc_sb)
                nc.sync.dma_start(y_d[h, r], ytn)

            first = True
            for h in range(HPC):
                kt = ktp.tile([P, NR, 512], MMDT, name=f"kt{h}", tag="kt")
                for n in range(NR):
                    nc.sync.dma_start(kt[:, n, :], kT_d[h, n])
                qt = qtp.tile([P, NR, 512], MMDT, name=f"qt{h}", tag="qt")
                for n in range(NR):
                    nc.sync.dma_start(qt[:, n, :], qT_d[h, n])
                vt = vtp.tile([P, NS, D], MMDT, name=f"vt{h}", tag="vt")
                for n in range(NR):
                    nc.sync.dma_start(vt[:, 4 * n:4 * (n + 1), :],
                                      v_d[h][:, 4 * n:4 * (n + 1), :])
                if first:
                    # issue Wproj loads once the critical h0 streams are queued
                    for n in range(NN):
                        for hw in range(HPC):
                            nc.sync.dma_start(wpt[:, n, hw, :], wp_d[n, hw])
                    first = False
                for r in range(NR):
                    nj = 4 * (r + 1)
                    yt_ps = psY.tile([P, STQ], F32, name=f"yt{h}_{r}",
                                     tag="yt")
                    acc_ab = [accp.tile([P, STQ], MMDT,
                                        name=f"acc{h}_{r}_{i}", tag=f"acc{i}")
                              for i in range(2)]
                    state[(h, r)] = (acc_ab, yt_ps)
                    for j in range(nj):
                        st_ps = psX.tile([P, STQ], F32,
                                         name=f"st{h}_{r}_{j}", tag="x")
                        mm(st_ps,
                           lhsT=kt[:, j // 4, (j % 4) * P:(j % 4 + 1) * P],
                           rhs=qt[:, r, :], start=True, stop=True)
                        pt = ptp.tile([P, STQ], MMDT,
                                      name=f"pt{h}_{r}_{j}", tag="pt")
                        if j >= nj - 4:
                            tmp = tmpp.tile([P, STQ], F32,
                                            name=f"tm{h}_{r}_{j}", tag="tmp")
                            nc.vector.tensor_add(
                                tmp, st_ps, masks[j - (nj - 4)])
                            nc.scalar.activation(pt, tmp, Exp, scale=NORM)
                        else:
                            nc.scalar.activation(pt, st_ps, Exp, scale=NORM)
                        ptf = pt.bitcast(F32)
                        eng = nc.vector if j % 2 == 0 else nc.gpsimd
                        a = acc_ab[j % 2]
                        if j < 2:
                            eng.tensor_copy(a, ptf)
                        else:
                            eng.tensor_add(a, a.bitcast(F32), ptf)
                        mm(yt_ps, lhsT=vt[:, j, :], rhs=pt,
                           start=(j == 0), stop=(j == nj - 1))
                    pend.append((h, r))
                    if len(pend) >= 2:
                        finalize_a(pend[-2])
                    if len(pend) >= 3:
                        finalize_b(pend.pop(0))
            finalize_a(pend[-1])
            for key in pend:
                finalize_b(key)

            # ------------- phase 2b: out projection -------------
            for r in range(NR):
                ys = []
                for h in range(HPC):
                    y_t = ysp.tile([P, STQ], MMDT, name=f"ys{r}_{h}",
                                   tag="ys")
                    nc.sync.dma_start(y_t, y_d[h, r])
                    ys.append(y_t)
                for n in range(NN):
                    for qq in range(4):
                        o_ps = psX.tile([P, 512], F32,
                                        name=f"o{r}_{n}_{qq}", tag="x")
                        for h in range(HPC):
                            mm(o_ps, lhsT=ys[h][:, qq * P:(qq + 1) * P],
                               rhs=wpt[:, n, h, :],
                               start=(h == 0), stop=(h == HPC - 1))
                        ob_t = obp.tile([P, 512], F32,
                                        name=f"ob{r}_{n}_{qq}", tag="ob")
                        nc.scalar.copy(ob_t, o_ps)
                        nc.sync.dma_start(
                            o_d[r * STQ + qq * P:r * STQ + (qq + 1) * P,
                                n * 512:(n + 1) * 512], ob_t)

    nc.compile()
    return nc


_NC = None


def _get_program():
    global _NC
    if _NC is None:
        _NC = _build_program()
    return _NC


def _maybe_install_trace_shim():
    """Provide antenv.axon_hooks (NTFF profiling) if the image lacks it."""
    import types
    if "antenv.axon_hooks" in sys.modules:
        return
    try:
        from trn_agent_boot.trn_boot import _ntff_profile_via_ctypes
        hook = _ntff_profile_via_ctypes("/opt/axon/libaxon_pjrt.so")
    except Exception:
        return
    mod = types.ModuleType("antenv.axon_hooks")
    mod.get_axon_ntff_profile_hook = lambda: hook
    mod.set_axon_ntff_profile_hook = lambda h: None
    sys.modules["antenv.axon_hooks"] = mod


def kernel(x, W_attn, b_attn, W_proj, b_proj):
    x = np.ascontiguousarray(np.asarray(x, dtype=np.float32))
    W_attn = np.ascontiguousarray(np.asarray(W_attn, dtype=np.float32))
    b_attn = np.ascontiguousarray(np.asarray(b_attn, dtype=np.float32))
    W_proj = np.ascontiguousarray(np.asarray(W_proj, dtype=np.float32))
    b_proj = np.ascontiguousarray(np.asarray(b_proj, dtype=np.float32))

    nc = _get_program()

    # per-parity weight shards (heads p*8 .. p*8+8), packed per DMA layouts
    shards = []
    for p in range(2):
        cs = slice(p * HPC * D, (p + 1) * HPC * D)

        def pack_hcpd(w):
            # [E, HPC*D] -> [HPC, P, NE, D]
            return np.ascontiguousarray(
                w.reshape(NE, P, HPC, D).transpose(2, 1, 0, 3))

        wq = W_attn[:, 0 * E:1 * E][:, cs]
        wk = W_attn[:, 1 * E:2 * E][:, cs]
        wv = W_attn[:, 2 * E:3 * E][:, cs]
        wp = W_proj[cs, :]
        shards.append({
            "wq": pack_hcpd(wq),
            "wk": pack_hcpd(wk),
            # [E, 1024] -> [2, NE, P, 512]
            "wv": np.ascontiguousarray(
                wv.reshape(NE, P, 2, 512).transpose(2, 0, 1, 3)),
            # [1024, E] -> [NN, HPC, P, 512]
            "wp": np.ascontiguousarray(
                wp.reshape(HPC, P, NN, 512).transpose(2, 0, 1, 3)),
            "bq": np.ascontiguousarray(
                b_attn[0 * E:1 * E][cs].reshape(HPC, D).T),
        })
    xTs = [np.ascontiguousarray(x[b].T) for b in range(B)]

    in_maps = []
    for core in range(8):
        b, p = core // 2, core % 2
        m = {"xT": xTs[b]}
        m.update(shards[p])
        in_maps.append(m)

    trace = bool(os.environ.get("CK_TRACE"))
    if trace:
        _maybe_install_trace_shim()
    res = run_bass_kernel_spmd(nc, in_maps, core_ids=list(range(8)),
                               trace=trace)
    if trace:
        kernel.last_exec_time_ns = res.exec_time_ns
        kernel.last_trace = res.instructions_and_trace

    # ------- host-side gather -------
    b_k = b_attn[1 * E:2 * E]
    b_v = b_attn[2 * E:3 * E]

    out = np.empty((B, S, E), dtype=np.float32)
    k_full = np.empty((B, H, S, D), dtype=np.float32)
    v_full = np.empty((B, H, S, D), dtype=np.float32)
    bias_out = (b_v @ W_proj + b_proj).astype(np.float32)

    for bi in range(B):
        r0, r1 = res.results[2 * bi], res.results[2 * bi + 1]
        out[bi] = r0["out_p"] + r1["out_p"] + bias_out[None, :]
        for p, r in ((0, r0), (1, r1)):
            kt = r["kT_out"]
            vv = r["v_out"]
            for j in range(HPC):
                h = p * HPC + j
                # [NR, P(d), 512(s)] -> [d, S] -> [S, d]
                k_full[bi, h] = (kt[j].transpose(1, 0, 2).reshape(D, S).T
                                 + b_k[h * D:(h + 1) * D][None, :])
                # [P(s), NS, D] -> [S, D]
                v_full[bi, h] = (vv[j].transpose(1, 0, 2).reshape(S, D)
                                 + b_v[h * D:(h + 1) * D][None, :])

    return out, k_full, v_full


# revision 24
# speedup vs baseline: 1.1082x; 1.1082x over previous
"""Causal self-attention (B=4, S=2048, E=2048, H=16, D=128) on 8 TRN2 cores.

Sharding: batch (4-way) x head-halves (2-way) -> 8 cores.
Core c handles batch b = c//2, heads p*8..p*8+8 where p = c%2.

Per-core kernel (single NEFF, SPMD):
  Phase 1: QKV projection.  x^T resident in SBUF [E,S]; per head computes
           q^T, k^T in [D,S] layout (matmul lhsT = W columns, rhs = x^T) and
           v in [S,D] layout (lhsT = x^T tile, rhs = W_v columns).
           q^T -> DRAM scratch, k^T -> output, v -> output.
  Phase 2: attention + proj in one pass; Wproj resident, k^T / v / q^T
           streamed per (q-tile, head).
           Scores computed transposed: ST[k,q] = (kT chunk) lhsT vs qT rhs.
           exp on scalar engine (no max subtraction needed: scores ~ N(0,1)),
           row sums accumulated on the vector engine + one ones-vector
           matmul, y^T via lhsT=v rhs=P^T, normalization via PE
           outer-product broadcast of 1/sums, out = y @ Wproj accumulated
           over all 8 heads in PSUM.  Normalization work is software-
           pipelined two heads deep to keep it off PE's critical path.
Host: shards inputs, sums the 2 partial outs per batch, fixes up k/v biases.

All DRAM tensors are laid out so DMA transfers are large contiguous blocks
(weights pre-packed on host into [head, chunk, 128, width] form).

Matmul inputs use float32r (single-pass PE, mantissa rounded to 12 bits)
unless CK_MM_DT=f32 (2-pass full fp32, ~4x slower).
"""

import math
import os
import sys

for _p in ("/opt/trn_rl_repo",):
    if os.path.isdir(_p) and _p not in sys.path:
        sys.path.append(_p)

import numpy as np

import concourse.bacc as bacc
import concourse.mybir as mybir
import concourse.tile as tile
from concourse.bass_utils import run_bass_kernel_spmd

B, S, E, H = 4, 2048, 2048, 16
D = E // H            # 128
P = 128               # partitions
HPC = H // 2          # heads per core = 8
NE = E // P           # 16 e-chunks
NS = S // P           # 16 s-chunks
STQ = 512             # phase-2 q tile width
NR = S // STQ         # 4 q tiles
NN = E // 512         # 4 output col chunks
NORM = 1.0 / math.sqrt(D)
NEG = -1.0e30

F32 = mybir.dt.float32
F32R = mybir.dt.float32r

MMDT = {"f32": F32, "f32r": F32R}[os.environ.get("CK_MM_DT", "f32r")]


def _build_program():
    nc = bacc.Bacc("TRN2", target_bir_lowering=False, debug=False)
    Exp = mybir.ActivationFunctionType.Exp
    Ident = mybir.ActivationFunctionType.Identity
    mm = nc.tensor.matmul

    with tile.TileContext(nc) as tc, \
         tc.tile_pool(name="dram", bufs=1, space="DRAM") as dram, \
         tc.tile_pool(name="const", bufs=1) as cp:

        def din(name, shape, dt=MMDT):
            return dram.tile(shape, dt, kind="ExternalInput", name=name,
                             uniquify=False)

        def dout(name, shape, dt=F32):
            return dram.tile(shape, dt, kind="ExternalOutput", name=name,
                             uniquify=False)

        xT_d = din("xT", [E, S])
        wq_d = din("wq", [HPC, P, NE, D])
        wk_d = din("wk", [HPC, P, NE, D])
        wv_d = din("wv", [2, NE, P, 512])
        wp_d = din("wp", [NN, HPC, P, 512])
        bq_d = din("bq", [P, HPC], F32)
        kT_d = dout("kT_out", [HPC, NR, P, 512], MMDT)
        v_d = dout("v_out", [HPC, P, NS, D], MMDT)
        o_d = dout("out_p", [S, E])
        qT_d = dram.tile([HPC, NR, P, 512], MMDT, kind="Internal",
                         name="qT_s", uniquify=False)

        # ---------------- constants ----------------
        masks = []
        for m in range(4):
            mk = cp.tile([P, STQ], F32, name=f"mask{m}", tag=f"mask{m}")
            nc.gpsimd.memset(mk, 0.0)
            # ST layout [k, q]: keep where q - k - 128*m >= 0
            nc.gpsimd.affine_select(
                out=mk, in_=mk, compare_op=mybir.AluOpType.is_ge,
                fill=NEG, base=-P * m, channel_multiplier=-1,
                pattern=[[1, STQ]])
            masks.append(mk)
        ones_col_f = cp.tile([P, 1], F32, name="ones_col_f", tag="oncf")
        nc.gpsimd.memset(ones_col_f, 1.0)
        ones_col = cp.tile([P, 1], MMDT, name="ones_col", tag="onc")
        nc.scalar.copy(ones_col, ones_col_f)
        ones_row_f = cp.tile([1, P], F32, name="ones_row_f", tag="onrf")
        nc.gpsimd.memset(ones_row_f, 1.0)
        ones_row = cp.tile([1, P], MMDT, name="ones_row", tag="onr")
        nc.scalar.copy(ones_row, ones_row_f)
        bq_sb = cp.tile([P, HPC], F32, name="bq_sb", tag="bq")
        nc.sync.dma_start(bq_sb, bq_d)

        # ---------------- phase 1: QKV ----------------
        with tc.tile_pool(name="p1", bufs=1) as p1, \
             tc.tile_pool(name="psP", bufs=8, space="PSUM") as psP:
            xt = p1.tile([P, NE, S], MMDT, name="xt", tag="xt")
            for c in range(NE):
                nc.sync.dma_start(xt[:, c, :], xT_d[c * P:(c + 1) * P, :])

            # q^T and k^T per head, interleaved per e-chunk
            with tc.tile_pool(name="ws", bufs=3) as ws, \
                 tc.tile_pool(name="evq", bufs=3) as evp:
                for h in range(HPC):
                    wtq = ws.tile([P, NE, D], MMDT, name=f"wq{h}", tag="w")
                    nc.sync.dma_start(wtq, wq_d[h])
                    wtk = ws.tile([P, NE, D], MMDT, name=f"wk{h}", tag="w")
                    nc.sync.dma_start(wtk, wk_d[h])
                    psq = [psP.tile([P, 512], F32, name=f"psq{h}_{n}",
                                    tag="ps") for n in range(4)]
                    psk = [psP.tile([P, 512], F32, name=f"psk{h}_{n}",
                                    tag="ps") for n in range(4)]
                    for c in range(NE):
                        for n in range(4):
                            mm(psq[n], lhsT=wtq[:, c, :],
                               rhs=xt[:, c, n * 512:(n + 1) * 512],
                               start=(c == 0), stop=(c == NE - 1))
                        for n in range(4):
                            mm(psk[n], lhsT=wtk[:, c, :],
                               rhs=xt[:, c, n * 512:(n + 1) * 512],
                               start=(c == 0), stop=(c == NE - 1))
                    evq_t = evp.tile([P, NR, 512], MMDT,
                                     name=f"evq{h}", tag="ev")
                    evk_t = evp.tile([P, NR, 512], MMDT,
                                     name=f"evk{h}", tag="ev")
                    for n in range(4):
                        nc.scalar.activation(evq_t[:, n, :], psq[n], Ident,
                                             bias=bq_sb[:, h:h + 1], scale=1.0)
                        nc.scalar.copy(evk_t[:, n, :], psk[n])
                    nc.sync.dma_start(
                        qT_d[h].rearrange("n p s -> p n s"), evq_t)
                    nc.sync.dma_start(
                        kT_d[h].rearrange("n p s -> p n s"), evk_t)

            # v in [S, 4*D] groups of 4 heads (wv resident per group)
            with tc.tile_pool(name="wvp", bufs=18) as wvp, \
                 tc.tile_pool(name="evv", bufs=3) as evvp:
                for g4 in range(2):
                    wvt = []
                    for c in range(NE):
                        wvc = wvp.tile([P, 512], MMDT,
                                       name=f"wv{g4}_{c}", tag="wv")
                        nc.sync.dma_start(wvc, wv_d[g4, c])
                        wvt.append(wvc)
                    for st in range(NS):
                        pv = psP.tile([P, 512], F32,
                                      name=f"pv{g4}_{st}", tag="ps")
                        for c in range(NE):
                            mm(pv, lhsT=xt[:, c, st * P:(st + 1) * P],
                               rhs=wvt[c], start=(c == 0), stop=(c == NE - 1))
                        ev = evvp.tile([P, 512], MMDT,
                                       name=f"evv{g4}_{st}", tag="evv")
                        nc.scalar.copy(ev, pv)
                        for hh in range(4):
                            nc.sync.dma_start(
                                v_d[g4 * 4 + hh][:, st, :],
                                ev[:, hh * D:(hh + 1) * D])

        # ---------------- phase 2a: attention (head-outer) ----------------
        y_d = dram.tile([HPC, NR, P, 512], MMDT, kind="Internal",
                        name="y_s", uniquify=False)
        from contextlib import ExitStack
        with ExitStack() as es:
            pool = lambda nm, bufs, **kw: es.enter_context(
                tc.tile_pool(name=nm, bufs=bufs, **kw))
            wpp = pool("wpp", 1); ktp = pool("ktp", 2); vtp = pool("vtp", 2)
            qtp = pool("qtp", 2); ptp = pool("ptp", 6); tmpp = pool("tmpp", 2)
            accp = pool("accp", 2); bcp = pool("bcp", 2); smp = pool("smp", 2)
            ytev = pool("ytev", 3); ysp = pool("ysp", 8); obp = pool("obp", 2)
            psX = pool("psX", 3, space="PSUM")
            psY = pool("psY", 3, space="PSUM")
            psS = pool("psS", 2, space="PSUM")

            wpt = wpp.tile([P, NN, HPC, 512], MMDT, name="wpt", tag="wp")

            state = {}
            pend = []

            def finalize_a(key):
                acc_ab, yt_ps = state[key]
                sm_ps = psS.tile([1, STQ], F32, name=f"sm_{key}", tag="sm")
                mm(sm_ps, lhsT=ones_col, rhs=acc_ab[0], start=True, stop=False)
                mm(sm_ps, lhsT=ones_col, rhs=acc_ab[1], start=False, stop=True)
                sm_sb = smp.tile([1, STQ], MMDT, name=f"smb_{key}", tag="smb")
                nc.scalar.copy(sm_sb, sm_ps)
                state[key] = (yt_ps, sm_sb)

            def finalize_b(key):
                h, r = key
                yt_ps, sm_sb = state.pop(key)
                bc_ps = psX.tile([P, STQ], F32, name=f"bcp_{key}", tag="x")
                mm(bc_ps, lhsT=ones_row, rhs=sm_sb, start=True, stop=True)
                bc_sb = bcp.tile([P, STQ], F32, name=f"bc_{key}", tag="bc")
                nc.vector.reciprocal(bc_sb, bc_ps)
                ytn = ytev.tile([P, STQ], MMDT, name=f"ytn_{key}", tag="ytn")
                nc.vector.tensor_mul(ytn, yt_ps, bc_sb)
                nc.sync.dma_start(y_d[h, r], ytn)

            first = True
            for h in range(HPC):
                kt = ktp.tile([P, NR, 512], MMDT, name=f"kt{h}", tag="kt")
                for n in range(NR):
                    nc.sync.dma_start(kt[:, n, :], kT_d[h, n])
                qt = qtp.tile([P, NR, 512], MMDT, name=f"qt{h}", tag="qt")
                for n in range(NR):
                    nc.sync.dma_start(qt[:, n, :], qT_d[h, n])
                vt = vtp.tile([P, NS, D], MMDT, name=f"vt{h}", tag="vt")
                for n in range(NR):
                    nc.sync.dma_start(vt[:, 4 * n:4 * (n + 1), :],
                                      v_d[h][:, 4 * n:4 * (n + 1), :])
                if first:
                    # issue Wproj loads once the critical h0 streams are queued
                    for n in range(NN):
                        for hw in range(HPC):
                            nc.sync.dma_start(wpt[:, n, hw, :], wp_d[n, hw])
                    first = False
                for r in range(NR):
                    nj = 4 * (r + 1)
                    yt_ps = psY.tile([P, STQ], F32, name=f"yt{h}_{r}",
                                     tag="yt")
                    acc_ab = [accp.tile([P, STQ], MMDT,
                                        name=f"acc{h}_{r}_{i}", tag=f"acc{i}")
                              for i in range(2)]
                    state[(h, r)] = (acc_ab, yt_ps)
                    for j in range(nj):
                        st_ps = psX.tile([P, STQ], F32,
                                         name=f"st{h}_{r}_{j}", tag="x")
                        mm(st_ps,
                           lhsT=kt[:, j // 4, (j % 4) * P:(j % 4 + 1) * P],
                           rhs=qt[:, r, :], start=True, stop=True)
                        pt = ptp.tile([P, STQ], MMDT,
                                      name=f"pt{h}_{r}_{j}", tag="pt")
                        if j >= nj - 4:
                            tmp = tmpp.tile([P, STQ], F32,
                                            name=f"tm{h}_{r}_{j}", tag="tmp")
                            nc.vector.tensor_add(
                                tmp, st_ps, masks[j - (nj - 4)])
                            nc.scalar.activation(pt, tmp, Exp, scale=NORM)
                        else:
                            nc.scalar.activation(pt, st_ps, Exp, scale=NORM)
                        ptf = pt.bitcast(F32)
                        eng = nc.vector if j % 2 == 0 else nc.gpsimd
                        a = acc_ab[j % 2]
                        if j < 2:
                            eng.tensor_copy(a, ptf)
                        else:
                            eng.tensor_add(a, a.bitcast(F32), ptf)
                        mm(yt_ps, lhsT=vt[:, j, :], rhs=pt,
                           start=(j == 0), stop=(j == nj - 1))
                    pend.append((h, r))
                    if len(pend) >= 2:
                        finalize_a(pend[-2])
                    if len(pend) >= 3:
                        finalize_b(pend.pop(0))
            finalize_a(pend[-1])
            for key in pend:
                finalize_b(key)

            # ------------- phase 2b: out projection -------------
            for r in range(NR):
                ys = []
                for h in range(HPC):
                    y_t = ysp.tile([P, STQ], MMDT, name=f"ys{r}_{h}",
                                   tag="ys")
                    nc.sync.dma_start(y_t, y_d[h, r])
                    ys.append(y_t)
                for n in range(NN):
                    ob_t = obp.tile([P, 4, 512], F32,
                                    name=f"ob{r}_{n}", tag="ob")
                    for qq in range(4):
                        o_ps = psX.tile([P, 512], F32,
                                        name=f"o{r}_{n}_{qq}", tag="x")
                        for h in range(HPC):
                            mm(o_ps, lhsT=ys[h][:, qq * P:(qq + 1) * P],
                               rhs=wpt[:, n, h, :],
                               start=(h == 0), stop=(h == HPC - 1))
                        nc.scalar.copy(ob_t[:, qq, :], o_ps)
                    nc.sync.dma_start(
                        o_d[r * STQ:(r + 1) * STQ,
                            n * 512:(n + 1) * 512].rearrange(
                                "(qq p) s -> p qq s", p=P),
                        ob_t)

    nc.compile()
    return nc


_NC = None


def _get_program():
    global _NC
    if _NC is None:
        _NC = _build_program()
    return _NC


def _maybe_install_trace_shim():
    """Provide antenv.axon_hooks (NTFF profiling) if the image lacks it."""
    import types
    if "antenv.axon_hooks" in sys.modules:
        return
    try:
        from trn_agent_boot.trn_boot import _ntff_profile_via_ctypes
        hook = _ntff_profile_via_ctypes("/opt/axon/libaxon_pjrt.so")
    except Exception:
        return
    mod = types.ModuleType("antenv.axon_hooks")
    mod.get_axon_ntff_profile_hook = lambda: hook
    mod.set_axon_ntff_profile_hook = lambda h: None
    sys.modules["antenv.axon_hooks"] = mod


def kernel(x, W_attn, b_attn, W_proj, b_proj):
    x = np.ascontiguousarray(np.asarray(x, dtype=np.float32))
    W_attn = np.ascontiguousarray(np.asarray(W_attn, dtype=np.float32))
    b_attn = np.ascontiguousarray(np.asarray(b_attn, dtype=np.float32))
    W_proj = np.ascontiguousarray(np.asarray(W_proj, dtype=np.float32))
    b_proj = np.ascontiguousarray(np.asarray(b_proj, dtype=np.float32))

    nc = _get_program()

    # per-parity weight shards (heads p*8 .. p*8+8), packed per DMA layouts
    shards = []
    for p in range(2):
        cs = slice(p * HPC * D, (p + 1) * HPC * D)

        def pack_hcpd(w):
            # [E, HPC*D] -> [HPC, P, NE, D]
            return np.ascontiguousarray(
                w.reshape(NE, P, HPC, D).transpose(2, 1, 0, 3))

        wq = W_attn[:, 0 * E:1 * E][:, cs]
        wk = W_attn[:, 1 * E:2 * E][:, cs]
        wv = W_attn[:, 2 * E:3 * E][:, cs]
        wp = W_proj[cs, :]
        shards.append({
            "wq": pack_hcpd(wq),
            "wk": pack_hcpd(wk),
            # [E, 1024] -> [2, NE, P, 512]
            "wv": np.ascontiguousarray(
                wv.reshape(NE, P, 2, 512).transpose(2, 0, 1, 3)),
            # [1024, E] -> [NN, HPC, P, 512]
            "wp": np.ascontiguousarray(
                wp.reshape(HPC, P, NN, 512).transpose(2, 0, 1, 3)),
            "bq": np.ascontiguousarray(
                b_attn[0 * E:1 * E][cs].reshape(HPC, D).T),
        })
    xTs = [np.ascontiguousarray(x[b].T) for b in range(B)]

    in_maps = []
    for core in range(8):
        b, p = core // 2, core % 2
        m = {"xT": xTs[b]}
        m.update(shards[p])
        in_maps.append(m)

    trace = bool(os.environ.get("CK_TRACE"))
    if trace:
        _maybe_install_trace_shim()
    res = run_bass_kernel_spmd(nc, in_maps, core_ids=list(range(8)),
                               trace=trace)
    if trace:
        kernel.last_exec_time_ns = res.exec_time_ns
        kernel.last_trace = res.instructions_and_trace

    # ------- host-side gather -------
    b_k = b_attn[1 * E:2 * E]
    b_v = b_attn[2 * E:3 * E]

    out = np.empty((B, S, E), dtype=np.float32)
    k_full = np.empty((B, H, S, D), dtype=np.float32)
    v_full = np.empty((B, H, S, D), dtype=np.float32)
    bias_out = (b_v @ W_proj + b_proj).astype(np.float32)

    for bi in range(B):
        r0, r1 = res.results[2 * bi], res.results[2 * bi + 1]
        out[bi] = r0["out_p"] + r1["out_p"] + bias_out[None, :]
        for p, r in ((0, r0), (1, r1)):
            kt = r["kT_out"]
            vv = r["v_out"]
            for j in range(HPC):
                h = p * HPC + j
                # [NR, P(d), 512(s)] -> [d, S] -> [S, d]
                k_full[bi, h] = (kt[j].transpose(1, 0, 2).reshape(D, S).T
                                 + b_k[h * D:(h + 1) * D][None, :])
                # [P(s), NS, D] -> [S, D]
                v_full[bi, h] = (vv[j].transpose(1, 0, 2).reshape(S, D)
                                 + b_v[h * D:(h + 1) * D][None, :])

    return out, k_full, v_full


# revision 27
# speedup vs baseline: 1.1183x; 1.0091x over previous
"""Causal self-attention (B=4, S=2048, E=2048, H=16, D=128) on 8 TRN2 cores.

Sharding: batch (4-way) x head-halves (2-way) -> 8 cores.
Core c handles batch b = c//2, heads p*8..p*8+8 where p = c%2.

Per-core kernel (single NEFF, SPMD):
  Phase 1: QKV projection.  x^T resident in SBUF [E,S]; per head computes
           q^T, k^T in [D,S] layout (matmul lhsT = W columns, rhs = x^T) and
           v in [S,D] layout (lhsT = x^T tile, rhs = W_v columns).
           q^T -> DRAM scratch, k^T -> output, v -> output.
  Phase 2: attention + proj in one pass; Wproj resident, k^T / v / q^T
           streamed per (q-tile, head).
           Scores computed transposed: ST[k,q] = (kT chunk) lhsT vs qT rhs.
           exp on scalar engine (no max subtraction needed: scores ~ N(0,1)),
           row sums accumulated on the vector engine + one ones-vector
           matmul, y^T via lhsT=v rhs=P^T, normalization via PE
           outer-product broadcast of 1/sums, out = y @ Wproj accumulated
           over all 8 heads in PSUM.  Normalization work is software-
           pipelined two heads deep to keep it off PE's critical path.
Host: shards inputs, sums the 2 partial outs per batch, fixes up k/v biases.

All DRAM tensors are laid out so DMA transfers are large contiguous blocks
(weights pre-packed on host into [head, chunk, 128, width] form).

Matmul inputs use float32r (single-pass PE, mantissa rounded to 12 bits)
unless CK_MM_DT=f32 (2-pass full fp32, ~4x slower).
"""

import math
import os
import sys

for _p in ("/opt/trn_rl_repo",):
    if os.path.isdir(_p) and _p not in sys.path:
        sys.path.append(_p)

import numpy as np

import concourse.bacc as bacc
import concourse.mybir as mybir
import concourse.tile as tile
from concourse.bass_utils import run_bass_kernel_spmd

B, S, E, H = 4, 2048, 2048, 16
D = E // H            # 128
P = 128               # partitions
HPC = H // 2          # heads per core = 8
NE = E // P           # 16 e-chunks
NS = S // P           # 16 s-chunks
STQ = 512             # phase-2 q tile width
NR = S // STQ         # 4 q tiles
NN = E // 512         # 4 output col chunks
NORM = 1.0 / math.sqrt(D)
NEG = -1.0e30

F32 = mybir.dt.float32
F32R = mybir.dt.float32r

MMDT = {"f32": F32, "f32r": F32R}[os.environ.get("CK_MM_DT", "f32r")]


def _build_program():
    nc = bacc.Bacc("TRN2", target_bir_lowering=False, debug=False)
    Exp = mybir.ActivationFunctionType.Exp
    Ident = mybir.ActivationFunctionType.Identity
    mm = nc.tensor.matmul

    with tile.TileContext(nc) as tc, \
         tc.tile_pool(name="dram", bufs=1, space="DRAM") as dram, \
         tc.tile_pool(name="const", bufs=1) as cp:

        def din(name, shape, dt=MMDT):
            return dram.tile(shape, dt, kind="ExternalInput", name=name,
                             uniquify=False)

        def dout(name, shape, dt=F32):
            return dram.tile(shape, dt, kind="ExternalOutput", name=name,
                             uniquify=False)

        xT_d = din("xT", [E, S])
        wq_d = din("wq", [HPC, P, NE, D])
        wk_d = din("wk", [HPC, P, NE, D])
        wv_d = din("wv", [2, NE, P, 512])
        wp_d = din("wp", [NN, HPC, P, 512])
        bq_d = din("bq", [P, HPC], F32)
        kT_d = dout("kT_out", [HPC, NR, P, 512], MMDT)
        v_d = dout("v_out", [HPC, P, NS, D], MMDT)
        o_d = dout("out_p", [S, E])
        qT_d = dram.tile([HPC, NR, P, 512], MMDT, kind="Internal",
                         name="qT_s", uniquify=False)

        # ---------------- constants ----------------
        masks = []
        for m in range(4):
            mk = cp.tile([P, STQ], F32, name=f"mask{m}", tag=f"mask{m}")
            nc.gpsimd.memset(mk, 0.0)
            # ST layout [k, q]: keep where q - k - 128*m >= 0
            nc.gpsimd.affine_select(
                out=mk, in_=mk, compare_op=mybir.AluOpType.is_ge,
                fill=NEG, base=-P * m, channel_multiplier=-1,
                pattern=[[1, STQ]])
            masks.append(mk)
        ones_col_f = cp.tile([P, 1], F32, name="ones_col_f", tag="oncf")
        nc.gpsimd.memset(ones_col_f, 1.0)
        ones_col = cp.tile([P, 1], MMDT, name="ones_col", tag="onc")
        nc.scalar.copy(ones_col, ones_col_f)
        ones_row_f = cp.tile([1, P], F32, name="ones_row_f", tag="onrf")
        nc.gpsimd.memset(ones_row_f, 1.0)
        ones_row = cp.tile([1, P], MMDT, name="ones_row", tag="onr")
        nc.scalar.copy(ones_row, ones_row_f)
        bq_sb = cp.tile([P, HPC], F32, name="bq_sb", tag="bq")
        nc.sync.dma_start(bq_sb, bq_d)

        # ---------------- phase 1: QKV ----------------
        with tc.tile_pool(name="p1", bufs=1) as p1, \
             tc.tile_pool(name="psP", bufs=8, space="PSUM") as psP:
            xt = p1.tile([P, NE, S], MMDT, name="xt", tag="xt")
            for c in range(NE):
                nc.sync.dma_start(xt[:, c, :], xT_d[c * P:(c + 1) * P, :])

            # q^T and k^T per head, interleaved per e-chunk
            with tc.tile_pool(name="ws", bufs=3) as ws, \
                 tc.tile_pool(name="evq", bufs=3) as evp:
                for h in range(HPC):
                    wtq = ws.tile([P, NE, D], MMDT, name=f"wq{h}", tag="w")
                    nc.sync.dma_start(wtq, wq_d[h])
                    wtk = ws.tile([P, NE, D], MMDT, name=f"wk{h}", tag="w")
                    nc.sync.dma_start(wtk, wk_d[h])
                    psq = [psP.tile([P, 512], F32, name=f"psq{h}_{n}",
                                    tag="ps") for n in range(4)]
                    psk = [psP.tile([P, 512], F32, name=f"psk{h}_{n}",
                                    tag="ps") for n in range(4)]
                    for c in range(NE):
                        for n in range(4):
                            mm(psq[n], lhsT=wtq[:, c, :],
                               rhs=xt[:, c, n * 512:(n + 1) * 512],
                               start=(c == 0), stop=(c == NE - 1))
                        for n in range(4):
                            mm(psk[n], lhsT=wtk[:, c, :],
                               rhs=xt[:, c, n * 512:(n + 1) * 512],
                               start=(c == 0), stop=(c == NE - 1))
                    evq_t = evp.tile([P, NR, 512], MMDT,
                                     name=f"evq{h}", tag="ev")
                    evk_t = evp.tile([P, NR, 512], MMDT,
                                     name=f"evk{h}", tag="ev")
                    for n in range(4):
                        nc.scalar.activation(evq_t[:, n, :], psq[n], Ident,
                                             bias=bq_sb[:, h:h + 1], scale=1.0)
                        nc.scalar.copy(evk_t[:, n, :], psk[n])
                    nc.sync.dma_start(
                        qT_d[h].rearrange("n p s -> p n s"), evq_t)
                    nc.sync.dma_start(
                        kT_d[h].rearrange("n p s -> p n s"), evk_t)

            # v in [S, 4*D] groups of 4 heads (wv resident per group)
            with tc.tile_pool(name="wvp", bufs=18) as wvp, \
                 tc.tile_pool(name="evv", bufs=3) as evvp:
                for g4 in range(2):
                    wvt = []
                    for c in range(NE):
                        wvc = wvp.tile([P, 512], MMDT,
                                       name=f"wv{g4}_{c}", tag="wv")
                        nc.sync.dma_start(wvc, wv_d[g4, c])
                        wvt.append(wvc)
                    for st in range(NS):
                        pv = psP.tile([P, 512], F32,
                                      name=f"pv{g4}_{st}", tag="ps")
                        for c in range(NE):
                            mm(pv, lhsT=xt[:, c, st * P:(st + 1) * P],
                               rhs=wvt[c], start=(c == 0), stop=(c == NE - 1))
                        ev = evvp.tile([P, 512], MMDT,
                                       name=f"evv{g4}_{st}", tag="evv")
                        nc.scalar.copy(ev, pv)
                        for hh in range(4):
                            nc.sync.dma_start(
                                v_d[g4 * 4 + hh][:, st, :],
                                ev[:, hh * D:(hh + 1) * D])

        # ---------------- phase 2a: attention (head-outer) ----------------
        y_d = dram.tile([HPC, NR, P, 512], MMDT, kind="Internal",
                        name="y_s", uniquify=False)
        from contextlib import ExitStack
        with ExitStack() as es:
            pool = lambda nm, bufs, **kw: es.enter_context(
                tc.tile_pool(name=nm, bufs=bufs, **kw))
            wpp = pool("wpp", 1); ktp = pool("ktp", 2); vtp = pool("vtp", 2)
            qtp = pool("qtp", 2); ptp = pool("ptp", 6); tmpp = pool("tmpp", 2)
            accp = pool("accp", 2); bcp = pool("bcp", 2); smp = pool("smp", 2)
            ytev = pool("ytev", 3); ysp = pool("ysp", 8); obp = pool("obp", 2)
            psX = pool("psX", 3, space="PSUM")
            psY = pool("psY", 3, space="PSUM")
            psS = pool("psS", 2, space="PSUM")

            wpt = wpp.tile([P, NN, HPC, 512], MMDT, name="wpt", tag="wp")

            state = {}
            pend = []

            def finalize_a(key):
                acc_ab, yt_ps = state[key]
                sm_ps = psS.tile([1, STQ], F32, name=f"sm_{key}", tag="sm")
                mm(sm_ps, lhsT=ones_col, rhs=acc_ab[0], start=True, stop=False)
                mm(sm_ps, lhsT=ones_col, rhs=acc_ab[1], start=False, stop=True)
                sm_sb = smp.tile([1, STQ], MMDT, name=f"smb_{key}", tag="smb")
                nc.scalar.copy(sm_sb, sm_ps)
                state[key] = (yt_ps, sm_sb)

            def finalize_b(key):
                h, r = key
                yt_ps, sm_sb = state.pop(key)
                bc_ps = psX.tile([P, STQ], F32, name=f"bcp_{key}", tag="x")
                mm(bc_ps, lhsT=ones_row, rhs=sm_sb, start=True, stop=True)
                bc_sb = bcp.tile([P, STQ], F32, name=f"bc_{key}", tag="bc")
                nc.vector.reciprocal(bc_sb, bc_ps)
                ytn = ytev.tile([P, STQ], MMDT, name=f"ytn_{key}", tag="ytn")
                nc.vector.tensor_mul(ytn, yt_ps, bc_sb)
                nc.sync.dma_start(y_d[h, r], ytn)

            first = True
            for h in range(HPC):
                kt = ktp.tile([P, NR, 512], MMDT, name=f"kt{h}", tag="kt")
                for n in range(NR):
                    nc.sync.dma_start(kt[:, n, :], kT_d[h, n])
                qt = qtp.tile([P, NR, 512], MMDT, name=f"qt{h}", tag="qt")
                nc.sync.dma_start(qt, qT_d[h].rearrange("n p s -> p n s"))
                vt = vtp.tile([P, NS, D], MMDT, name=f"vt{h}", tag="vt")
                nc.sync.dma_start(vt, v_d[h])
                if first:
                    # issue Wproj loads once the critical h0 streams are queued
                    for n in range(NN):
                        for hw in range(HPC):
                            nc.sync.dma_start(wpt[:, n, hw, :], wp_d[n, hw])
                    first = False
                for r in range(NR):
                    nj = 4 * (r + 1)
                    yt_ps = psY.tile([P, STQ], F32, name=f"yt{h}_{r}",
                                     tag="yt")
                    acc_ab = [accp.tile([P, STQ], MMDT,
                                        name=f"acc{h}_{r}_{i}", tag=f"acc{i}")
                              for i in range(2)]
                    state[(h, r)] = (acc_ab, yt_ps)
                    for j in range(nj):
                        st_ps = psX.tile([P, STQ], F32,
                                         name=f"st{h}_{r}_{j}", tag="x")
                        mm(st_ps,
                           lhsT=kt[:, j // 4, (j % 4) * P:(j % 4 + 1) * P],
                           rhs=qt[:, r, :], start=True, stop=True)
                        pt = ptp.tile([P, STQ], MMDT,
                                      name=f"pt{h}_{r}_{j}", tag="pt")
                        if j >= nj - 4:
                            tmp = tmpp.tile([P, STQ], F32,
                                            name=f"tm{h}_{r}_{j}", tag="tmp")
                            nc.vector.tensor_add(
                                tmp, st_ps, masks[j - (nj - 4)])
                            nc.scalar.activation(pt, tmp, Exp, scale=NORM)
                        else:
                            nc.scalar.activation(pt, st_ps, Exp, scale=NORM)
                        ptf = pt.bitcast(F32)
                        eng = nc.vector if j % 2 == 0 else nc.gpsimd
                        a = acc_ab[j % 2]
                        if j < 2:
                            eng.tensor_copy(a, ptf)
                        else:
                            eng.tensor_add(a, a.bitcast(F32), ptf)
                        mm(yt_ps, lhsT=vt[:, j, :], rhs=pt,
                           start=(j == 0), stop=(j == nj - 1))
                    pend.append((h, r))
                    if len(pend) >= 2:
                        finalize_a(pend[-2])
                    if len(pend) >= 3:
                        finalize_b(pend.pop(0))
            finalize_a(pend[-1])
            for key in pend:
                finalize_b(key)

            # ------------- phase 2b: out projection -------------
            for r in range(NR):
                ys = []
                for h in range(HPC):
                    y_t = ysp.tile([P, STQ], MMDT, name=f"ys{r}_{h}",
                                   tag="ys")
                    nc.sync.dma_start(y_t, y_d[h, r])
                    ys.append(y_t)
                for n in range(NN):
                    ob_t = obp.tile([P, 4, 512], F32,
                                    name=f"ob{r}_{n}", tag="ob")
                    for qq in range(4):
                        o_ps = psX.tile([P, 512], F32,
                                        name=f"o{r}_{n}_{qq}", tag="x")
                        for h in range(HPC):
                            mm(o_ps, lhsT=ys[h][:, qq * P:(qq + 1) * P],
                               rhs=wpt[:, n, h, :],
                               start=(h == 0), stop=(h == HPC - 1))
                        nc.scalar.copy(ob_t[:, qq, :], o_ps)
                    nc.sync.dma_start(
                        o_d[r * STQ:(r + 1) * STQ,
                            n * 512:(n + 1) * 512].rearrange(
                                "(qq p) s -> p qq s", p=P),
                        ob_t)

    nc.compile()
    return nc


_NC = None


def _get_program():
    global _NC
    if _NC is None:
        _NC = _build_program()
    return _NC


def _maybe_install_trace_shim():
    """Provide antenv.axon_hooks (NTFF profiling) if the image lacks it."""
    import types
    if "antenv.axon_hooks" in sys.modules:
        return
    try:
        from trn_agent_boot.trn_boot import _ntff_profile_via_ctypes
        hook = _ntff_profile_via_ctypes("/opt/axon/libaxon_pjrt.so")
    except Exception:
        return
    mod = types.ModuleType("antenv.axon_hooks")
    mod.get_axon_ntff_profile_hook = lambda: hook
    mod.set_axon_ntff_profile_hook = lambda h: None
    sys.modules["antenv.axon_hooks"] = mod


def kernel(x, W_attn, b_attn, W_proj, b_proj):
    x = np.ascontiguousarray(np.asarray(x, dtype=np.float32))
    W_attn = np.ascontiguousarray(np.asarray(W_attn, dtype=np.float32))
    b_attn = np.ascontiguousarray(np.asarray(b_attn, dtype=np.float32))
    W_proj = np.ascontiguousarray(np.asarray(W_proj, dtype=np.float32))
    b_proj = np.ascontiguousarray(np.asarray(b_proj, dtype=np.float32))

    nc = _get_program()

    # per-parity weight shards (heads p*8 .. p*8+8), packed per DMA layouts
    shards = []
    for p in range(2):
        cs = slice(p * HPC * D, (p + 1) * HPC * D)

        def pack_hcpd(w):
            # [E, HPC*D] -> [HPC, P, NE, D]
            return np.ascontiguousarray(
                w.reshape(NE, P, HPC, D).transpose(2, 1, 0, 3))

        wq = W_attn[:, 0 * E:1 * E][:, cs]
        wk = W_attn[:, 1 * E:2 * E][:, cs]
        wv = W_attn[:, 2 * E:3 * E][:, cs]
        wp = W_proj[cs, :]
        shards.append({
            "wq": pack_hcpd(wq),
            "wk": pack_hcpd(wk),
            # [E, 1024] -> [2, NE, P, 512]
            "wv": np.ascontiguousarray(
                wv.reshape(NE, P, 2, 512).transpose(2, 0, 1, 3)),
            # [1024, E] -> [NN, HPC, P, 512]
            "wp": np.ascontiguousarray(
                wp.reshape(HPC, P, NN, 512).transpose(2, 0, 1, 3)),
            "bq": np.ascontiguousarray(
                b_attn[0 * E:1 * E][cs].reshape(HPC, D).T),
        })
    xTs = [np.ascontiguousarray(x[b].T) for b in range(B)]

    in_maps = []
    for core in range(8):
        b, p = core // 2, core % 2
        m = {"xT": xTs[b]}
        m.update(shards[p])
        in_maps.append(m)

    trace = bool(os.environ.get("CK_TRACE"))
    if trace:
        _maybe_install_trace_shim()
    res = run_bass_kernel_spmd(nc, in_maps, core_ids=list(range(8)),
                               trace=trace)
    if trace:
        kernel.last_exec_time_ns = res.exec_time_ns
        kernel.last_trace = res.instructions_and_trace

    # ------- host-side gather -------
    b_k = b_attn[1 * E:2 * E]
    b_v = b_attn[2 * E:3 * E]

    out = np.empty((B, S, E), dtype=np.float32)
    k_full = np.empty((B, H, S, D), dtype=np.float32)
    v_full = np.empty((B, H, S, D), dtype=np.float32)
    bias_out = (b_v @ W_proj + b_proj).astype(np.float32)

    for bi in range(B):
        r0, r1 = res.results[2 * bi], res.results[2 * bi + 1]
        out[bi] = r0["out_p"] + r1["out_p"] + bias_out[None, :]
        for p, r in ((0, r0), (1, r1)):
            kt = r["kT_out"]
            vv = r["v_out"]
            for j in range(HPC):
                h = p * HPC + j
                # [NR, P(d), 512(s)] -> [d, S] -> [S, d]
                k_full[bi, h] = (kt[j].transpose(1, 0, 2).reshape(D, S).T
                                 + b_k[h * D:(h + 1) * D][None, :])
                # [P(s), NS, D] -> [S, D]
                v_full[bi, h] = (vv[j].transpose(1, 0, 2).reshape(S, D)
                                 + b_v[h * D:(h + 1) * D][None, :])

    return out, k_full, v_full
